# revision 1
# baseline (speedup 1.0000x reference)
"""Deformable head attention kernel for 8 Trainium2 NeuronCores.

Sharding: core i handles batch b = i//2 and head-group hg = i%2 (heads
4*hg..4*hg+3, all 4096 queries). The reference's final reshape maps output
pixel p' to head p'//512's features of queries 8t..8t+7 (t = p' % 512), so a
head-group owns output rows [hg*32, hg*32+32) exactly -- fully local per core.

Per core:
  - channel-major matmuls on PE (q/offset/attention/key projections, output)
  - softmax + bilinear coordinate/weight math on DVE/ACT in [q-partition] layout
  - per-(head,scale) quad maps (2x2x32ch tokens, 256B bf16) in DRAM
  - SWDGE dma_gather fetches one quad per sample; DVE does the weighted reduce
  - output scramble via a DRAM round-trip, then the Wm projection
"""
import os
import numpy as np
from contextlib import ExitStack

import concourse.bass as bass
import concourse.tile as tile
from concourse import bacc, mybir
from concourse.bass_utils import run_bass_kernel_spmd
from concourse.masks import make_identity

F32 = mybir.dt.float32
I32 = mybir.dt.int32
I16 = mybir.dt.int16
BF16 = mybir.dt.bfloat16
OP = mybir.AluOpType
AF = mybir.ActivationFunctionType

HEADS, KPTS, SCALES, D = 8, 4, 4, 256
DK = D // HEADS              # 32
HL = 4                       # heads per core
B, H, W = 4, 64, 64
Q = 4096                     # queries per core (full image)
QC = Q // 128                # 32 q-chunks
HW_SIZES = [(16, 16), (32, 32), (64, 64), (128, 128)]
POS = [h * w for h, w in HW_SIZES]
TCAP = [p + w + 4 for p, (h, w) in zip(POS, HW_SIZES)]
NCORES = 8
NIDX = KPTS * 2048           # 8192 gather indices per (head, scale, q-half)

_cache = {}
PH = os.environ.get("KPH", "IDEF")


def _build():
    nc = bacc.Bacc("TRN2", target_bir_lowering=False, debug=False)

    d_queryT = nc.dram_tensor("queryT", [2, 128, Q], F32, kind="ExternalInput")
    d_keysT = [nc.dram_tensor(f"keysT{l}", [2, 128, POS[l]], F32, kind="ExternalInput")
               for l in range(SCALES)]
    d_refx = nc.dram_tensor("refx", [128, QC], F32, kind="ExternalInput")
    d_refy = nc.dram_tensor("refy", [128, QC], F32, kind="ExternalInput")
    d_Wq = nc.dram_tensor("Wq", [2, 128, D], F32, kind="ExternalInput")
    d_Wk = nc.dram_tensor("Wk", [2, 128, 128], F32, kind="ExternalInput")
    d_Woff = nc.dram_tensor("WoffP", [2, 128, 128], F32, kind="ExternalInput")
    d_WA = nc.dram_tensor("WA", [2, 128, 64], F32, kind="ExternalInput")
    d_Wm = nc.dram_tensor("Wm", [2, 128, D], F32, kind="ExternalInput")
    d_bq = nc.dram_tensor("bq", [2, 128, 1], F32, kind="ExternalInput")
    d_bk = nc.dram_tensor("bk", [128, 1], F32, kind="ExternalInput")
    d_bm = nc.dram_tensor("bm", [2, 128, 1], F32, kind="ExternalInput")
    d_boff = nc.dram_tensor("boffE", [128, 128], F32, kind="ExternalInput")
    d_bA = nc.dram_tensor("bAE", [128, 64], F32, kind="ExternalInput")
    cnames = ["wl_t", "wlm1_t", "wlm2_t", "wlp1_t", "hlm1_t", "hlm2_t",
              "hl_t", "cofx_t", "cofy_t"]
    d_const = {n: nc.dram_tensor(n, [128, 64], F32, kind="ExternalInput")
               for n in cnames}

    d_out = nc.dram_tensor("outT", [2, 128, 2048], F32, kind="ExternalOutput")
    d_map = [nc.dram_tensor(f"map{l}", [HL, TCAP[l], 128], BF16)
             for l in range(SCALES)]
    d_featD = nc.dram_tensor("featD", [2048, 256], F32)   # scrambled [o, c']

    with tile.TileContext(nc) as tc, ExitStack() as ctx:
        wpool = ctx.enter_context(tc.tile_pool(name="weights", bufs=1))
        ppool = ctx.enter_context(tc.tile_pool(name="persist", bufs=1))
        psum = ctx.enter_context(tc.tile_pool(name="psum", bufs=2, space="PSUM"))

        def load2(d, n, nm):
            t = [wpool.tile([128, n], F32, tag=f"{nm}{i}", name=f"{nm}{i}")
                 for i in range(2)]
            for i in range(2):
                nc.sync.dma_start(t[i][:], d[i])
            return t

        def load1(d, shape, nm):
            t = wpool.tile(shape, F32, tag=nm, name=nm)
            nc.sync.dma_start(t[:], d[:])
            return t

        Wq = load2(d_Wq, D, "Wq"); Wk = load2(d_Wk, 128, "Wk")
        Woff = load2(d_Woff, 128, "Woff"); WA = load2(d_WA, 64, "WA")
        Wm = load2(d_Wm, D, "Wm")
        bq = load2(d_bq, 1, "bq"); bm = load2(d_bm, 1, "bm")
        bk = load1(d_bk, [128, 1], "bk")
        boffE = load1(d_boff, [128, 128], "boffE")
        bAE = load1(d_bA, [128, 64], "bAE")
        C = {n: load1(d_const[n], [128, 64], n) for n in cnames}
        refx = load1(d_refx, [128, QC], "refx")
        refy = load1(d_refy, [128, QC], "refy")
        ident = wpool.tile([128, 128], F32, tag="ident", name="ident")
        make_identity(nc, ident[:])
        tok16 = wpool.tile([128, QC, 64], I16, tag="tok16", name="tok16")

        def bh(t, n=64):
            return t[:].rearrange("p (o f) -> p o f", o=1).broadcast_to([128, QC, n])

        W4 = ppool.tile([128, HL, SCALES, KPTS, QC, 4, 2], BF16, tag="W4", name="W4")
        feat = ppool.tile([128, QC, HL, DK], F32, tag="feat", name="feat")

        # zero map edge regions first (independent of everything else)
        with tc.tile_pool(name="zp", bufs=1) as zp:
            zt = zp.tile([128, 288], BF16, tag="zero", name="zero")
            nc.vector.memset(zt[:], 0)
            for l in range(SCALES):
                _, wl = HW_SIZES[l]
                dmv = d_map[l][:].rearrange("h t e -> h (t e)")
                ze1 = wl + 4
                ze2 = TCAP[l] - POS[l] + wl
                for h in range(HL):
                    nc.sync.dma_start(
                        dmv[h, 0:ze1 * 128].rearrange("(p f) -> p f", p=128),
                        zt[:, 0:ze1])
                    nc.sync.dma_start(
                        dmv[h, (POS[l] - wl) * 128:TCAP[l] * 128]
                            .rearrange("(p f) -> p f", p=128),
                        zt[:, 0:ze2])
        tc.strict_bb_all_engine_barrier()

        with tc.tile_pool(name="pbc", bufs=1) as pbc, \
             tc.tile_pool(name="poff", bufs=1) as poff:
            Aw = pbc.tile([128, QC, 64], F32, tag="Aw", name="Aw")
            offx = poff.tile([128, QC, 64], F32, tag="offx", name="offx")
            offy = poff.tile([128, QC, 64], F32, tag="offy", name="offy")

            # =========== phase B: projections ===========
            with tc.tile_pool(name="proj", bufs=1) as proj:
                queryT = [proj.tile([128, Q], F32, tag=f"qin{i}", name=f"qin{i}")
                          for i in range(2)]
                for i in range(2):
                    nc.sync.dma_start(queryT[i][:], d_queryT[i])
                qT = [proj.tile([128, Q], F32, tag=f"qT{i}", name=f"qT{i}")
                      for i in range(2)]
                for m in range(2):
                    for n in range(Q // 512):
                        ps = psum.tile([128, 512], F32, tag="mm", name="mm")
                        for k in range(2):
                            nc.tensor.matmul(ps[:], Wq[k][:, m * 128:(m + 1) * 128],
                                             queryT[k][:, n * 512:(n + 1) * 512],
                                             start=(k == 0), stop=(k == 1))
                        nc.scalar.activation(qT[m][:, n * 512:(n + 1) * 512], ps[:],
                                             AF.Identity, bias=bq[m][:], scale=1.0)
                for c in range(QC):
                    ps = psum.tile([128, 512], F32, tag="mm", name="mm")
                    for k in range(2):
                        nc.tensor.matmul(ps[:, 0:128], qT[k][:, c * 128:(c + 1) * 128],
                                         Woff[k][:], start=(k == 0), stop=(k == 1))
                    nc.scalar.activation(offx[:, c], ps[:, 0:64], AF.Copy)
                    nc.scalar.activation(offy[:, c], ps[:, 64:128], AF.Copy)
                    ps2 = psum.tile([128, 512], F32, tag="mm", name="mm")
                    for k in range(2):
                        nc.tensor.matmul(ps2[:, 0:64], qT[k][:, c * 128:(c + 1) * 128],
                                         WA[k][:], start=(k == 0), stop=(k == 1))
                    nc.scalar.activation(Aw[:, c], ps2[:, 0:64], AF.Copy)
                nc.vector.tensor_tensor(offx[:], offx[:], bh(boffE[:, 0:64]), OP.add)
                nc.vector.tensor_tensor(offy[:], offy[:], bh(boffE[:, 64:128]), OP.add)
                nc.vector.tensor_tensor(Aw[:], Aw[:], bh(bAE), OP.add)
                nc.scalar.activation(Aw[:], Aw[:], AF.Exp)
                Aw4 = Aw[:].rearrange("p c (h s) -> p c h s", s=16)
                ssum = pbc.tile([128, QC, HL], F32, tag="ssum", name="ssum")
                nc.vector.tensor_reduce(ssum[:], Aw4, mybir.AxisListType.X, OP.add)
                nc.vector.reciprocal(ssum[:], ssum[:])
                rb = ssum[:].rearrange("p c (h o) -> p c h o", o=1) \
                            .broadcast_to([128, QC, HL, 16])
                nc.vector.tensor_tensor(Aw4, Aw4, rb, OP.mult)

            # =========== phase C: coords, weights, tokens ===========
            with tc.tile_pool(name="coord", bufs=1) as cp:
                def ct(tag, dt=F32):
                    return cp.tile([128, QC, 64], dt, tag=tag, name=tag)

                ix = ct("ix"); iy = ct("iy")
                for c in range(QC):
                    nc.vector.tensor_scalar(ix[:, c], C["wl_t"][:], refx[:, c:c + 1],
                                            -0.5, OP.mult, OP.add)
                    nc.vector.tensor_scalar(iy[:, c], C["hl_t"][:], refy[:, c:c + 1],
                                            -0.5, OP.mult, OP.add)
                nc.vector.tensor_tensor(offx[:], offx[:], bh(C["cofx_t"]), OP.mult)
                nc.vector.tensor_tensor(offy[:], offy[:], bh(C["cofy_t"]), OP.mult)
                nc.vector.tensor_tensor(ix[:], ix[:], offx[:], OP.add)
                nc.vector.tensor_tensor(iy[:], iy[:], offy[:], OP.add)

                res = {}
                for dim, (iv, lim1, lim2) in enumerate(
                        ((ix, C["wlm1_t"], C["wlm2_t"]),
                         (iy, C["hlm1_t"], C["hlm2_t"]))):
                    xm = ct("xm")
                    nc.vector.tensor_scalar(xm[:], iv[:], 0.5, None, OP.subtract)
                    xi = ct("xi", I32)
                    nc.vector.tensor_copy(xi[:], xm[:])      # RNE => floor(iv)
                    x0 = ct(f"x0_{dim}")
                    nc.vector.tensor_copy(x0[:], xi[:])
                    w1 = ct(f"w1_{dim}")
                    nc.vector.tensor_tensor(w1[:], iv[:], x0[:], OP.subtract)
                    w0 = ct(f"w0_{dim}")
                    nc.vector.tensor_scalar(w0[:], w1[:], -1.0, 1.0, OP.mult, OP.add)
                    m = ct("mA")
                    mb = ct("mB")
                    nc.vector.tensor_scalar(m[:], x0[:], 0.0, None, OP.is_ge)
                    nc.vector.tensor_tensor(mb[:], x0[:], bh(lim1), OP.is_le)
                    nc.vector.tensor_tensor(m[:], m[:], mb[:], OP.mult)
                    nc.vector.tensor_tensor(w0[:], w0[:], m[:], OP.mult)
                    nc.vector.tensor_scalar(m[:], x0[:], -1.0, None, OP.is_ge)
                    nc.vector.tensor_tensor(mb[:], x0[:], bh(lim2), OP.is_le)
                    nc.vector.tensor_tensor(m[:], m[:], mb[:], OP.mult)
                    nc.vector.tensor_tensor(w1[:], w1[:], m[:], OP.mult)
                    res[dim] = (x0, w0, w1)
                x0f, wx0, wx1 = res[0]
                y0f, wy0, wy1 = res[1]

                W4v = W4[:].rearrange("p h s k c f d -> p c (h s k) f d")
                u = ct("mA")
                for cy, wyv in ((0, wy0), (1, wy1)):
                    nc.vector.tensor_tensor(u[:], Aw[:], wyv[:], OP.mult)
                    for cx, wxv in ((0, wx0), (1, wx1)):
                        for dup in range(2):
                            nc.vector.tensor_tensor(
                                W4v[:, :, :, cy * 2 + cx, dup],
                                u[:], wxv[:], OP.mult)

                nc.vector.tensor_scalar(y0f[:], y0f[:], -1.0, None, OP.max)
                nc.vector.tensor_tensor(y0f[:], y0f[:], bh(C["hlm1_t"]), OP.min)
                nc.vector.tensor_scalar(x0f[:], x0f[:], -1.0, None, OP.max)
                nc.vector.tensor_tensor(x0f[:], x0f[:], bh(C["wl_t"]), OP.min)
                tokf = ct("xm")
                nc.vector.tensor_tensor(tokf[:], y0f[:], bh(C["wl_t"]), OP.mult)
                nc.vector.tensor_tensor(tokf[:], tokf[:], x0f[:], OP.add)
                nc.vector.tensor_tensor(tokf[:], tokf[:], bh(C["wlp1_t"]), OP.add)
                toki = ct("xi", I32)
                nc.vector.tensor_copy(toki[:], tokf[:])
                nc.vector.tensor_copy(tok16[:], toki[:].bitcast(I16)[:, :, 0:128:2])

        # wrapped gather-index layout, chunk = (qh, hl, s):
        # IDXW[p%16, qh, hl, s, k, qc, p//16] = tok16[p, qh*16+qc, (hl,s,k)]
        idxp = ctx.enter_context(tc.tile_pool(name="idxp", bufs=1))
        IDXW = idxp.tile([128, 2, HL, SCALES, KPTS, 16, 8], I16,
                         tag="IDXW", name="IDXW")
        for ph in range(8 if "I" in PH else 0):
            for qcg in range(QC):
                qh, qcl = qcg // 16, qcg % 16
                eng = nc.sync if (ph + qcg) % 2 == 0 else nc.scalar
                eng.dma_start(IDXW[0:16, qh, :, :, :, qcl, ph]
                              .rearrange("p h s k -> p (h s k)"),
                              tok16[ph * 16:(ph + 1) * 16, qcg, :])
        for d0, n in ((16, 16), (32, 32), (64, 64)):
            nc.sync.dma_start(IDXW[d0:d0 + n], IDXW[0:n])

        # =========== phase D: quad maps ===========
        if "D" in PH:
          with tc.tile_pool(name="mapp", bufs=3) as mp:
            for l in range(SCALES):
                hl_, wl = HW_SIZES[l]
                dmv = d_map[l][:].rearrange("h t e -> h (t e)")
                nslab = max(POS[l] // 512, 1)
                slab = POS[l] // nslab
                SUP = min(4, nslab)          # matmul slabs per slot-write
                stg = None
                for sl in range(nslab):
                    if sl % SUP == 0:
                        stg = mp.tile([128, 4 * SUP, 128], BF16, tag="stg",
                                      name="stg")
                    kin = [mp.tile([128, slab], F32, tag=f"kin{i}", name=f"kin{i}")
                           for i in range(2)]
                    for i in range(2):
                        nc.sync.dma_start(kin[i][:],
                                          d_keysT[l][i, :, sl * slab:(sl + 1) * slab])
                    kfs = mp.tile([128, slab], F32, tag="kfs", name="kfs")
                    ps = psum.tile([128, 512], F32, tag="mm", name="mm")
                    for k in range(2):
                        nc.tensor.matmul(ps[:, 0:slab], Wk[k][:],
                                         kin[k][:], start=(k == 0), stop=(k == 1))
                    nc.scalar.activation(kfs[:], ps[:, 0:slab], AF.Identity,
                                         bias=bk[:], scale=1.0)
                    nsub = slab // 128
                    for sub in range(nsub):
                        pt = psum.tile([128, 128], F32, tag="tp", name="tp")
                        nc.tensor.transpose(
                            pt[:], kfs[:, sub * 128:(sub + 1) * 128], ident[:])
                        nc.scalar.activation(stg[:, (sl % SUP) * nsub + sub], pt[:],
                                             AF.Copy)
                    if sl % SUP == SUP - 1:
                        sup0 = (sl // SUP) * SUP
                        supsz = SUP * slab
                        src_ap = stg[:, 0:SUP * nsub] \
                            .rearrange("p a (hl c) -> p hl a c", c=DK)
                        for cy in range(2):
                            for cx in range(2):
                                base = sup0 * slab + wl + 1 - cy * wl - cx
                                start = base * 128 + (cy * 2 + cx) * DK
                                dst4 = dmv[:, start:start + supsz * 128] \
                                    .rearrange("h (t e) -> h t e", e=128)[:, :, 0:DK] \
                                    .rearrange("h (a p) e -> p h a e", p=128)
                                for hh in range(HL):
                                    eng = nc.sync if hh % 2 == 0 else nc.scalar
                                    eng.dma_start(dst4[:, hh], src_ap[:, hh])

        tc.strict_bb_all_engine_barrier()

        # =========== phase E: gather + interpolate ===========
        nc.vector.memset(feat[:], 0)
        if "E" in PH:
          with tc.tile_pool(name="gath", bufs=1) as gp:
            for h in range(HL):
                for l in range(SCALES):
                    for qh in range(2):
                        G = gp.tile([128, KPTS * 16, 128], BF16, tag="G", name="G",
                                    bufs=2)
                        nc.gpsimd.dma_gather(
                            G[:], d_map[l][h],
                            IDXW[:, qh, h, l].rearrange("p k c e -> p (k c e)"),
                            num_idxs=NIDX, num_idxs_reg=NIDX,
                            elem_size=128, elem_step=128, single_packet=False)
                        M = gp.tile([128, KPTS * 16, 128], BF16, tag="M", name="M")
                        for kk in range(KPTS):
                            wv = W4[:, h, l, kk, qh * 16:(qh + 1) * 16, :, :] \
                                .rearrange("p c f (o d) -> p c f o d", o=1) \
                                .broadcast_to([128, 16, 4, 16, 2])
                            nc.vector.tensor_tensor(
                                M[:, kk * 16:(kk + 1) * 16]
                                    .rearrange("p c (f a b) -> p c f a b", a=16, b=2),
                                G[:, kk * 16:(kk + 1) * 16]
                                    .rearrange("p c (f a b) -> p c f a b", a=16, b=2),
                                wv, OP.mult)
                        r1 = gp.tile([128, KPTS * 16, 64], BF16, tag="r1", name="r1")
                        nc.vector.tensor_tensor(r1[:], M[:, :, 0:64], M[:, :, 64:128],
                                                OP.add)
                        r2 = gp.tile([128, KPTS * 16, DK], BF16, tag="r2", name="r2")
                        nc.vector.tensor_tensor(r2[:], r1[:, :, 0:32], r1[:, :, 32:64],
                                                OP.add)
                        t1 = gp.tile([128, 2 * 16, DK], BF16, tag="t1", name="t1")
                        nc.vector.tensor_tensor(t1[:], r2[:, 0:32], r2[:, 32:64],
                                                OP.add)
                        t2 = gp.tile([128, 16, DK], F32, tag="t2", name="t2")
                        nc.vector.tensor_tensor(t2[:], t1[:, 0:16], t1[:, 16:32],
                                                OP.add)
                        fslice = feat[:, qh * 16:(qh + 1) * 16, h]
                        nc.vector.tensor_tensor(fslice, fslice, t2[:], OP.add)

        # =========== phase F: scramble via DRAM + output projection ===========
        # featD[o, c'] = feat[q=8t+e, h, d] with o = h*512 + t, c' = e*32 + d
        fD = d_featD[:].rearrange("o c -> (o c)")
        for e in range(8):
            for h in range(HL):
                src = feat[e:128:8, :, h, :]
                dst = bass.AP(fD.tensor,
                              fD.offset + h * 512 * 256 + e * DK,
                              ((256, 16), (16 * 256, QC), (1, DK)))
                eng = nc.sync if (e + h) % 2 == 0 else nc.scalar
                eng.dma_start(dst, src)

        tc.strict_bb_all_engine_barrier()

        with tc.tile_pool(name="outp", bufs=1) as op:
            INq = op.tile([128, 16, 256], F32, tag="INq", name="INq")
            nc.sync.dma_start(
                INq[:], d_featD[:].rearrange("(a p) c -> p a c", p=128))
            featT = [op.tile([128, 2048], F32, tag=f"fT{i}", name=f"fT{i}")
                     for i in range(2)]
            for ch in range(16):
                for m in range(2):
                    pt = psum.tile([128, 128], F32, tag="tp", name="tp")
                    nc.tensor.transpose(pt[:], INq[:, ch, m * 128:(m + 1) * 128],
                                        ident[:])
                    nc.scalar.activation(featT[m][:, ch * 128:(ch + 1) * 128], pt[:],
                                         AF.Copy)
            outT = [op.tile([128, 2048], F32, tag=f"oT{i}", name=f"oT{i}")
                    for i in range(2)]
            for m in range(2):
                for n in range(2048 // 512):
                    ps = psum.tile([128, 512], F32, tag="mm", name="mm")
                    for k in range(2):
                        nc.tensor.matmul(ps[:], Wm[k][:, m * 128:(m + 1) * 128],
                                         featT[k][:, n * 512:(n + 1) * 512],
                                         start=(k == 0), stop=(k == 1))
                    nc.scalar.activation(outT[m][:, n * 512:(n + 1) * 512], ps[:],
                                         AF.Identity, bias=bm[m][:], scale=1.0)
                nc.sync.dma_start(d_out[m], outT[m][:])

    nc.compile()
    return nc


def _prep_inputs(query, keys, ref_point, Wq, bq, Wk, bk, Woff, boff, WA, bA, Wm, bm):
    def two(w, n):
        return np.ascontiguousarray(w.reshape(2, 128, n).astype(np.float32))

    wl_arr = np.zeros(64, np.float32)
    hl_arr = np.zeros(64, np.float32)
    for h in range(HL):
        for s in range(SCALES):
            for k in range(KPTS):
                hl_, wl_ = HW_SIZES[s]
                wl_arr[h * 16 + s * 4 + k] = wl_
                hl_arr[h * 16 + s * 4 + k] = hl_
    consts = {
        "wl_t": np.tile(wl_arr, (128, 1)),
        "wlm1_t": np.tile(wl_arr - 1, (128, 1)),
        "wlm2_t": np.tile(wl_arr - 2, (128, 1)),
        "wlp1_t": np.tile(wl_arr + 1, (128, 1)),
        "hlm1_t": np.tile(hl_arr - 1, (128, 1)),
        "hlm2_t": np.tile(hl_arr - 2, (128, 1)),
        "hl_t": np.tile(hl_arr, (128, 1)),
        "cofx_t": np.tile(wl_arr / (wl_arr - 1), (128, 1)),
        "cofy_t": np.tile(hl_arr / (hl_arr - 1), (128, 1)),
    }
    consts = {k: np.ascontiguousarray(v.astype(np.float32)) for k, v in consts.items()}

    rs = ref_point.reshape(Q, 2)
    refx = np.ascontiguousarray(rs[:, 0].reshape(QC, 128).T)
    refy = np.ascontiguousarray(rs[:, 1].reshape(QC, 128).T)

    in_maps = []
    for core in range(NCORES):
        b, hg = core // 2, core % 2
        heads = range(4 * hg, 4 * hg + 4)
        perm_off = np.zeros(128, np.int64)
        perm_A = np.zeros(64, np.int64)
        for i, h in enumerate(heads):
            for s in range(SCALES):
                for k in range(KPTS):
                    for xy in range(2):
                        perm_off[xy * 64 + i * 16 + s * 4 + k] = \
                            ((h * SCALES + s) * KPTS + k) * 2 + xy
                    perm_A[i * 16 + s * 4 + k] = (h * SCALES + s) * KPTS + k
        WoffP = np.ascontiguousarray(Woff[:, perm_off])
        boffP = boff[perm_off]
        WAP = np.ascontiguousarray(WA[:, perm_A])
        bAP = bA[perm_A]
        chs = slice(4 * hg * DK, (4 * hg + 4) * DK)
        m = {
            "Wq": two(Wq, D), "Wk": two(np.ascontiguousarray(Wk[:, chs]), 128),
            "WoffP": two(WoffP, 128), "WA": two(WAP, 64), "Wm": two(Wm, D),
            "bq": two(bq, 1), "bm": two(bm, 1),
            "bk": np.ascontiguousarray(bk[chs]).reshape(128, 1).astype(np.float32),
            "boffE": np.tile(boffP, (128, 1)).astype(np.float32),
            "bAE": np.tile(bAP, (128, 1)).astype(np.float32),
            "refx": refx, "refy": refy, **consts,
        }
        qs = query[b].reshape(Q, D)
        m["queryT"] = np.ascontiguousarray(qs.T).reshape(2, 128, Q)
        for l in range(SCALES):
            m[f"keysT{l}"] = np.ascontiguousarray(
                keys[l][b].reshape(POS[l], D).T).reshape(2, 128, POS[l])
        in_maps.append(m)
    return in_maps


def kernel(query, keys0, keys1, keys2, keys3, ref_point,
           Wq, bq, Wk, bk, Woff, boff, WA, bA, Wm, bm):
    query = np.asarray(query, np.float32)
    keys = [np.asarray(k, np.float32) for k in (keys0, keys1, keys2, keys3)]
    in_maps = _prep_inputs(
        query, keys, np.asarray(ref_point, np.float32),
        np.asarray(Wq, np.float32), np.asarray(bq, np.float32),
        np.asarray(Wk, np.float32), np.asarray(bk, np.float32),
        np.asarray(Woff, np.float32), np.asarray(boff, np.float32),
        np.asarray(WA, np.float32), np.asarray(bA, np.float32),
        np.asarray(Wm, np.float32), np.asarray(bm, np.float32))
    if "nc" not in _cache:
        _cache["nc"] = _build()
    nc = _cache["nc"]
    res = run_bass_kernel_spmd(nc, in_maps, list(range(NCORES)))
    out = np.zeros((B, H, W, D), np.float32)
    for core in range(NCORES):
        b, hg = core // 2, core % 2
        oT = res.results[core]["outT"].reshape(D, 2048)
        out[b, 32 * hg:32 * hg + 32] = oT.T.reshape(32, W, D)
    return out



# revision 12
# speedup vs baseline: 1.3337x; 1.3337x over previous
"""Deformable head attention kernel for 8 Trainium2 NeuronCores.

Sharding: core i handles batch b = i//2 and head-group hg = i%2 (heads
4*hg..4*hg+3, all 4096 queries). The reference's final reshape maps output
pixel p' to head p'//512's features of queries 8t..8t+7 (t = p' % 512), so a
head-group owns output rows [hg*32, hg*32+32) exactly -- fully local per core.

v2 layout (vs baseline):
  - all projections in bf16 (4x cheaper PE matmuls, half the load bytes)
  - quad maps built on-chip: 4 corner-shifted PE transposes per 128-texel
    block -> psum [slot, h, ch] -> ACT/DVE interleave to [h, slot, ch]
    rows -> 1024B-contiguous DMA writes (vs 64B-descriptor quadrant writes)
  - gather-index fold via permuted-identity PE transposes + a 2-stage DRAM
    round trip (vs 256 2-byte-element DMAs)
  - scramble/output path in bf16 with h-merged access patterns
"""
import numpy as np
import ml_dtypes
from contextlib import ExitStack

import concourse.bass as bass
import concourse.tile as tile
from concourse import bacc, mybir
from concourse.bass_utils import run_bass_kernel_spmd

F32 = mybir.dt.float32
I32 = mybir.dt.int32
I16 = mybir.dt.int16
BF16 = mybir.dt.bfloat16
OP = mybir.AluOpType
AF = mybir.ActivationFunctionType
BF = ml_dtypes.bfloat16

HEADS, KPTS, SCALES, D = 8, 4, 4, 256
DK = D // HEADS              # 32
HL = 4                       # heads per core
B, H, W = 4, 64, 64
Q = 4096                     # queries per core (full image)
QC = Q // 128                # 32 q-chunks
HW_SIZES = [(16, 16), (32, 32), (64, 64), (128, 128)]
POS = [h * w for h, w in HW_SIZES]
NRB = [(p + w + 4 + 127) // 128 for p, (h, w) in zip(POS, HW_SIZES)]  # 3,9,33,130
PADL = 136
NCORES = 8

# f32 const blob columns
CC = {}
_c = 0
for _n in ["wl", "hl", "hlm1", "wlp1", "cofx", "cofy", "bcox", "bcoy", "bA"]:
    CC[_n] = _c
    _c += 64
CC["refx"] = _c; _c += QC
CC["refy"] = _c; _c += QC
for _n in ["bq0", "bq1", "bk", "bm0", "bm1", "cneghalf", "cone"]:
    CC[_n] = _c
    _c += 1
NCC = _c

# bf16 const blob columns (mask limits)
CB = {"wlm1": 0, "wlm2": 64, "hlm1": 128, "hlm2": 192}
NCB = 256

# weight blob columns (per contraction half): Wq | Wk | Woff | WA | Wm
WB = {"Wq": 0, "Wk": 256, "Woff": 384, "WA": 512, "Wm": 576}
NWB = 832

_cache = {}


def _build():
    nc = bacc.Bacc("TRN2", target_bir_lowering=False, debug=False)

    d_queryT = nc.dram_tensor("queryT", [2, 128, Q], BF16, kind="ExternalInput")
    d_keysT = [nc.dram_tensor(f"keysT{l}", [2, 128, POS[l]], BF16,
                              kind="ExternalInput") for l in range(SCALES)]
    d_wblob = nc.dram_tensor("wblob", [2, 128, NWB], BF16, kind="ExternalInput")
    d_cblob = nc.dram_tensor("cblob", [128, NCC], F32, kind="ExternalInput")
    d_cb16 = nc.dram_tensor("cb16", [128, NCB], BF16, kind="ExternalInput")
    d_identb = nc.dram_tensor("identb", [128, 128], BF16, kind="ExternalInput")
    d_pmat = nc.dram_tensor("pmat", [128, 128], F32, kind="ExternalInput")

    d_out = nc.dram_tensor("outT", [2, 128, 2048], F32, kind="ExternalOutput")
    d_map = [nc.dram_tensor(f"map{l}", [NRB[l] * 128, 512], BF16)
             for l in range(SCALES)]
    d_idxA = nc.dram_tensor("idxA", [128, 2048], I16)     # L-layout scratch
    d_featD = nc.dram_tensor("featD", [2048, 256], BF16)  # scrambled [o, c']

    with tile.TileContext(nc) as tc, ExitStack() as ctx:
        wpool = ctx.enter_context(tc.tile_pool(name="weights", bufs=1))
        ppool = ctx.enter_context(tc.tile_pool(name="persist", bufs=1))
        psum = ctx.enter_context(tc.tile_pool(name="psum", bufs=2, space="PSUM"))

        wb = [wpool.tile([128, NWB], BF16, tag=f"wb{i}", name=f"wb{i}")
              for i in range(2)]
        for i in range(2):
            nc.sync.dma_start(wb[i][:], d_wblob[i])
        cb = wpool.tile([128, NCC], F32, tag="cb", name="cb")
        nc.sync.dma_start(cb[:], d_cblob[:])
        cb16 = wpool.tile([128, NCB], BF16, tag="cb16", name="cb16")
        nc.sync.dma_start(cb16[:], d_cb16[:])
        identb = wpool.tile([128, 128], BF16, tag="identb", name="identb")
        nc.sync.dma_start(identb[:], d_identb[:])
        pmat = wpool.tile([128, 128], F32, tag="pmat", name="pmat")
        nc.sync.dma_start(pmat[:], d_pmat[:])

        def cbc(nm, n=64):
            # [p, 1, n] -> broadcast over QC
            return cb[:, CC[nm]:CC[nm] + n].rearrange(
                "p (o f) -> p o f", o=1).broadcast_to([128, QC, n])

        def cb16c(nm):
            return cb16[:, CB[nm]:CB[nm] + 64].rearrange(
                "p (o f) -> p o f", o=1).broadcast_to([128, QC, 64])

        def bias(nm):
            return cb[:, CC[nm]:CC[nm] + 1]

        W4 = ppool.tile([128, HL, SCALES, KPTS, QC, 4, 2], BF16, tag="W4",
                        name="W4")
        feat = ppool.tile([128, QC, HL, DK], F32, tag="feat", name="feat")
        IDX16 = ppool.tile([128, HL, SCALES, 2, KPTS, 16, 8], I16, tag="IDX16",
                           name="IDX16")

        # =========== phase B: projections ===========
        pbc = ctx.enter_context(tc.tile_pool(name="pbc", bufs=1))
        Awb = pbc.tile([128, QC, 64], BF16, tag="Awb", name="Awb")
        offx = pbc.tile([128, QC, 64], F32, tag="offx", name="offx")
        offy = pbc.tile([128, QC, 64], F32, tag="offy", name="offy")

        with tc.tile_pool(name="proj", bufs=1) as proj:
            queryT = [proj.tile([128, Q], BF16, tag=f"qin{i}", name=f"qin{i}")
                      for i in range(2)]
            for i in range(2):
                nc.sync.dma_start(queryT[i][:], d_queryT[i])
            qT = [proj.tile([128, Q], BF16, tag=f"qT{i}", name=f"qT{i}")
                  for i in range(2)]
            for m in range(2):
                for n in range(Q // 512):
                    ps = psum.tile([128, 512], F32, tag="mm", name="mm")
                    for k in range(2):
                        nc.tensor.matmul(
                            ps[:], wb[k][:, WB["Wq"] + m * 128:WB["Wq"] + (m + 1) * 128],
                            queryT[k][:, n * 512:(n + 1) * 512],
                            start=(k == 0), stop=(k == 1))
                    nc.scalar.activation(qT[m][:, n * 512:(n + 1) * 512], ps[:],
                                         AF.Identity, bias=bias(f"bq{m}"),
                                         scale=1.0)

            Aw = pbc.tile([128, QC, 64], F32, tag="Aw", name="Aw")
            # offsets: psum groups of 4 q-chunks
            for g in range(QC // 4):
                ps = psum.tile([128, 512], F32, tag="mm", name="mm")
                for j in range(4):
                    c = g * 4 + j
                    for k in range(2):
                        nc.tensor.matmul(
                            ps[:, j * 128:(j + 1) * 128],
                            qT[k][:, c * 128:(c + 1) * 128],
                            wb[k][:, WB["Woff"]:WB["Woff"] + 128],
                            start=(k == 0), stop=(k == 1))
                psv = ps[:].rearrange("p (j x) -> p j x", j=4)
                nc.scalar.activation(offx[:, g * 4:(g + 1) * 4], psv[:, :, 0:64],
                                     AF.Copy)
                nc.scalar.activation(offy[:, g * 4:(g + 1) * 4], psv[:, :, 64:128],
                                     AF.Copy)
            # attention logits: psum groups of 8 q-chunks
            for g in range(QC // 8):
                ps = psum.tile([128, 512], F32, tag="mm", name="mm")
                for j in range(8):
                    c = g * 8 + j
                    for k in range(2):
                        nc.tensor.matmul(
                            ps[:, j * 64:(j + 1) * 64],
                            qT[k][:, c * 128:(c + 1) * 128],
                            wb[k][:, WB["WA"]:WB["WA"] + 64],
                            start=(k == 0), stop=(k == 1))
                nc.scalar.activation(Aw[:, g * 8:(g + 1) * 8],
                                     ps[:].rearrange("p (j x) -> p j x", j=8),
                                     AF.Copy)

            nc.vector.tensor_tensor(Aw[:], Aw[:], cbc("bA"), OP.add)
            nc.scalar.activation(Aw[:], Aw[:], AF.Exp)
            Aw4 = Aw[:].rearrange("p c (h s) -> p c h s", s=16)
            ssum = proj.tile([128, QC, HL], F32, tag="ssum", name="ssum")
            nc.vector.tensor_reduce(ssum[:], Aw4, mybir.AxisListType.X, OP.add)
            nc.vector.reciprocal(ssum[:], ssum[:])
            rb = ssum[:].rearrange("p c (h o) -> p c h o", o=1) \
                        .broadcast_to([128, QC, HL, 16])
            nc.vector.tensor_tensor(
                Awb[:].rearrange("p c (h s) -> p c h s", s=16), Aw4, rb, OP.mult)

        # =========== phase C: coords, weights, tok ===========
        with tc.tile_pool(name="coord", bufs=1) as cp:
            ixy = cp.tile([128, QC, 64], F32, tag="ixy", name="ixy")
            xm = cp.tile([128, QC, 64], F32, tag="xm", name="xm")
            xi = cp.tile([128, QC, 64], I32, tag="xi", name="xi")
            x0f = [cp.tile([128, QC, 64], F32, tag=f"x0f{d}", name=f"x0f{d}")
                   for d in range(2)]
            x0b = cp.tile([128, QC, 64], BF16, tag="x0b", name="x0b")
            wgt = {}
            for d in range(2):
                for w in range(2):
                    wgt[d, w] = cp.tile([128, QC, 64], BF16, tag=f"w{d}{w}",
                                        name=f"w{d}{w}")
            mA = cp.tile([128, QC, 64], BF16, tag="mA", name="mA")
            mB = cp.tile([128, QC, 64], BF16, tag="mB", name="mB")

            for d, (off, base, cof, bco, ref, lim1, lim2, clampw) in enumerate((
                    (offx, "wl", "cofx", "bcox", "refx", "wlm1", "wlm2", "wl"),
                    (offy, "hl", "cofy", "bcoy", "refy", "hlm1", "hlm2", "hlm1"))):

                refv = cb[:, CC[ref]:CC[ref] + QC].rearrange(
                    "p (c o) -> p c o", o=1).broadcast_to([128, QC, 64])
                # ix = wl*ref + off*cof + (boff*cof - 0.5)
                nc.vector.tensor_tensor(off[:], off[:], cbc(cof), OP.mult)
                nc.vector.tensor_tensor(off[:], off[:], cbc(bco), OP.add)
                nc.vector.tensor_tensor(ixy[:], cbc(base), refv, OP.mult)
                nc.vector.tensor_tensor(ixy[:], ixy[:], off[:], OP.add)
                # x0 = floor(ix) via RNE(ix - 0.5)
                nc.scalar.activation(xm[:], ixy[:], AF.Identity,
                                     bias=bias("cneghalf"), scale=1.0)
                nc.vector.tensor_copy(xi[:], xm[:])
                nc.vector.tensor_copy(x0f[d][:], xi[:])
                nc.vector.tensor_copy(x0b[:], xi[:])
                # w1 = ix - x0 ; w0 = 1 - w1   (bf16)
                w0v, w1v = wgt[d, 0], wgt[d, 1]
                nc.vector.tensor_tensor(w1v[:], ixy[:], x0f[d][:], OP.subtract)
                nc.scalar.activation(w0v[:], w1v[:], AF.Identity,
                                     bias=bias("cone"), scale=-1.0)
                # masks (bf16; x0 is integer-valued, exact in bf16)
                nc.vector.tensor_scalar(mA[:], x0b[:], 0.0, None, OP.is_ge)
                nc.vector.tensor_tensor(mB[:], x0b[:], cb16c(lim1), OP.is_le)
                nc.vector.tensor_tensor(mA[:], mA[:], mB[:], OP.mult)
                nc.vector.tensor_tensor(w0v[:], w0v[:], mA[:], OP.mult)
                nc.vector.tensor_scalar(mA[:], x0b[:], -1.0, None, OP.is_ge)
                nc.vector.tensor_tensor(mB[:], x0b[:], cb16c(lim2), OP.is_le)
                nc.vector.tensor_tensor(mA[:], mA[:], mB[:], OP.mult)
                nc.vector.tensor_tensor(w1v[:], w1v[:], mA[:], OP.mult)
                # clamp for token index
                nc.vector.tensor_scalar(x0f[d][:], x0f[d][:], -1.0, None, OP.max)
                nc.vector.tensor_tensor(x0f[d][:], x0f[d][:], cbc(clampw), OP.min)

            # W4[p, h, s, k, c, slot, dup] = A * wy_cy * wx_cx
            W4v = W4[:].rearrange("p h s k c f d -> p c (h s k) f d")
            u = cp.tile([128, QC, 64], BF16, tag="u", name="u")
            for cy in range(2):
                nc.vector.tensor_tensor(u[:], Awb[:], wgt[1, cy][:], OP.mult)
                for cx in range(2):
                    uv = u[:].rearrange("p c (f o) -> p c f o", o=1) \
                             .broadcast_to([128, QC, 64, 2])
                    wxv = wgt[0, cx][:].rearrange("p c (f o) -> p c f o", o=1) \
                                       .broadcast_to([128, QC, 64, 2])
                    nc.vector.tensor_tensor(W4v[:, :, :, cy * 2 + cx], uv, wxv,
                                            OP.mult)

            # tok = y0*wl + x0 + wl + 1  (f32, exact)
            tokf = xm
            nc.vector.tensor_tensor(tokf[:], x0f[1][:], cbc("wl"), OP.mult)
            nc.vector.tensor_tensor(tokf[:], tokf[:], x0f[0][:], OP.add)
            nc.vector.tensor_tensor(tokf[:], tokf[:], cbc("wlp1"), OP.add)

            # transpose tok (f32) with permuted identity: out col n = p'*8+e
            TT32 = cp.tile([128, 2048], I32, tag="TT32", name="TT32")
            tokfl = tokf[:].rearrange("p c h -> p (c h)")
            for g in range(4):
                pt = psum.tile([128, 512], F32, tag="tp", name="tp")
                for j in range(4):
                    b = g * 4 + j
                    nc.tensor.transpose(pt[:, j * 128:(j + 1) * 128],
                                        tokfl[:, b * 128:(b + 1) * 128], pmat[:])
                nc.vector.tensor_copy(TT32[:, g * 512:(g + 1) * 512], pt[:])
            TT16 = cp.tile([128, 2048], I16, tag="TT16", name="TT16")
            nc.vector.tensor_copy(TT16[:], TT32[:].bitcast(I16)[:, 0:4096:2])

            # DMA_A: TT16 [P=(qcp,hsk), b, n=(p',e)] -> L layout in DRAM
            fA = d_idxA[:].rearrange("a b -> (a b)")
            dstA = bass.AP(fA.tensor, fA.offset,
                           ((128, 128), (16384, 16), (1, 128)))
            nc.sync.dma_start(
                dstA, TT16[:].rearrange("p (b n) -> p b n", b=16))

        tc.strict_bb_all_engine_barrier()

        # =========== DMA_B: fold-read L -> IDX16, + replicate ===========
        fA = d_idxA[:].rearrange("a b -> (a b)")
        for hh in range(HL):
            for s in range(SCALES):
                for qh in range(2):
                    for k in range(KPTS):
                        srcB = bass.AP(
                            fA.tensor,
                            fA.offset + hh * 2048 + s * 512 + qh * 131072
                            + k * 128,
                            ((8, 16), (8192, 16), (1, 8)))
                        dst = IDX16[0:16, hh, s, qh, k].rearrange(
                            "p c e -> p (c e)")
                        eng = nc.sync if (hh * 8 + s * 2 + qh + k) % 2 == 0 \
                            else nc.scalar
                        eng.dma_start(dst, srcB)
        for dd0, n in ((16, 16), (32, 32), (64, 64)):
            nc.sync.dma_start(IDX16[dd0:dd0 + n], IDX16[0:n])

        # =========== phase D: per-scale kf + on-chip quad rows ===========
        with tc.tile_pool(name="mapp", bufs=1) as dp:
            for l in range(SCALES):
                wl = HW_SIZES[l][1]
                nrb = NRB[l]
                kf = dp.tile([128, PADL + NRB[3] * 128], BF16, tag="kf",
                             name="kf")
                nc.vector.memset(kf[:, 0:PADL], 0)
                nc.vector.memset(
                    kf[:, PADL + POS[l]:PADL + nrb * 128], 0)
                CHK = 4096
                for c0 in range(0, POS[l], CHK):
                    cw = min(CHK, POS[l] - c0)
                    kin = [dp.tile([128, CHK], BF16, tag=f"kin{i}",
                                   name=f"kin{i}", bufs=2) for i in range(2)]
                    for i in range(2):
                        nc.sync.dma_start(kin[i][:, 0:cw],
                                          d_keysT[l][i, :, c0:c0 + cw])
                    for s0 in range(0, cw, 512):
                        sw = min(512, cw - s0)
                        ps = psum.tile([128, 512], F32, tag="mm", name="mm")
                        for k in range(2):
                            nc.tensor.matmul(ps[:, 0:sw], wb[k][:, WB["Wk"]:WB["Wk"] + 128],
                                             kin[k][:, s0:s0 + sw],
                                             start=(k == 0), stop=(k == 1))
                        nc.scalar.activation(kf[:, PADL + c0 + s0:PADL + c0 + s0 + sw],
                                             ps[:, 0:sw], AF.Identity,
                                             bias=bias("bk"), scale=1.0)
                # corner transposes + interleave + row writes
                dmv = d_map[l][:].rearrange("r c -> (r c)")
                for g0 in range(0, nrb, 4):
                    gb = min(4, nrb - g0)
                    stg = dp.tile([128, 4, 512], BF16, tag="stg", name="stg",
                                  bufs=2)
                    for bi in range(gb):
                        rb = g0 + bi
                        pt = psum.tile([128, 512], BF16, tag="tpb", name="tpb")
                        for slot, dlt in enumerate((wl + 1, wl, 1, 0)):
                            w0 = PADL + rb * 128 - dlt
                            nc.tensor.transpose(pt[:, slot * 128:(slot + 1) * 128],
                                                kf[:, w0:w0 + 128], identb[:])
                        piv = pt[:].rearrange("p (f h x) -> p f h x", f=4, h=4)
                        sov = stg[:, bi].rearrange("p (h f x) -> p f h x",
                                                   h=4, f=4)
                        if rb % 2 == 0:
                            nc.scalar.activation(sov, piv, AF.Copy)
                        else:
                            nc.vector.tensor_copy(sov, piv)
                    dstM = bass.AP(dmv.tensor, dmv.offset + g0 * 128 * 512,
                                   ((512, 128), (65536, gb), (1, 512)))
                    eng = nc.sync if (g0 // 4) % 2 == 0 else nc.scalar
                    eng.dma_start(dstM, stg[:, 0:gb])

        nc.vector.memset(feat[:], 0)
        tc.strict_bb_all_engine_barrier()

        # =========== phase E: gather + interpolate ===========
        with tc.tile_pool(name="gath", bufs=1) as gp:
            for l in range(SCALES):
                map_f = d_map[l][:].rearrange("r c -> (r c)")
                for hh in range(HL):
                    in_ap = bass.AP(map_f.tensor, map_f.offset + hh * 128,
                                    ((512, NRB[l] * 128), (1, 128)))
                    for qh in range(2):
                        G = gp.tile([128, 64, 128], BF16, tag="G", name="G",
                                    bufs=2)
                        nc.gpsimd.dma_gather(
                            G[:], in_ap,
                            IDX16[:, hh, l, qh].rearrange("p k c e -> p (k c e)"),
                            num_idxs=8192, num_idxs_reg=8192,
                            elem_size=128, elem_step=512, single_packet=False)
                        M = gp.tile([128, 64, 128], BF16, tag="M", name="M")
                        for kk in range(KPTS):
                            wv = W4[:, hh, l, kk, qh * 16:(qh + 1) * 16] \
                                .rearrange("p c f (o d) -> p c f o d", o=1) \
                                .broadcast_to([128, 16, 4, 16, 2])
                            nc.vector.tensor_tensor(
                                M[:, kk * 16:(kk + 1) * 16]
                                    .rearrange("p c (f a b) -> p c f a b",
                                               a=16, b=2),
                                G[:, kk * 16:(kk + 1) * 16]
                                    .rearrange("p c (f a b) -> p c f a b",
                                               a=16, b=2),
                                wv, OP.mult)
                        r1 = gp.tile([128, 64, 64], BF16, tag="r1", name="r1")
                        nc.vector.tensor_tensor(r1[:], M[:, :, 0:64],
                                                M[:, :, 64:128], OP.add)
                        r2 = gp.tile([128, 64, 32], BF16, tag="r2", name="r2")
                        nc.vector.tensor_tensor(r2[:], r1[:, :, 0:32],
                                                r1[:, :, 32:64], OP.add)
                        t1 = gp.tile([128, 32, 32], BF16, tag="t1", name="t1")
                        nc.vector.tensor_tensor(t1[:], r2[:, 0:32], r2[:, 32:64],
                                                OP.add)
                        t2 = gp.tile([128, 16, 32], BF16, tag="t2", name="t2")
                        nc.vector.tensor_tensor(t2[:], t1[:, 0:16], t1[:, 16:32],
                                                OP.add)
                        fslice = feat[:, qh * 16:(qh + 1) * 16, hh]
                        nc.vector.tensor_tensor(fslice, fslice, t2[:], OP.add)

        # =========== phase F: scramble via DRAM + output projection ===========
        with tc.tile_pool(name="outp", bufs=1) as op:
            featb = op.tile([128, QC, HL, DK], BF16, tag="featb", name="featb")
            nc.vector.tensor_copy(featb[:], feat[:])
            fD = d_featD[:].rearrange("o c -> (o c)")
            for e in range(8):
                for hh in range(HL):
                    src = featb[e:128:8, :, hh]
                    dst = bass.AP(fD.tensor,
                                  fD.offset + e * DK + hh * 131072,
                                  ((256, 16), (4096, QC), (1, DK)))
                    eng = nc.sync if (e + hh) % 2 == 0 else nc.scalar
                    eng.dma_start(dst, src)

            tc.strict_bb_all_engine_barrier()

            INq = op.tile([128, 16, 256], BF16, tag="INq", name="INq")
            nc.sync.dma_start(
                INq[:], d_featD[:].rearrange("(a p) c -> p a c", p=128))
            featT = [op.tile([128, 2048], BF16, tag=f"fT{i}", name=f"fT{i}")
                     for i in range(2)]
            for m in range(2):
                for a0 in range(0, 16, 4):
                    pt = psum.tile([128, 512], BF16, tag="tpb", name="tpb")
                    for j in range(4):
                        nc.tensor.transpose(
                            pt[:, j * 128:(j + 1) * 128],
                            INq[:, a0 + j, m * 128:(m + 1) * 128], identb[:])
                    nc.scalar.activation(
                        featT[m][:, a0 * 128:(a0 + 4) * 128], pt[:], AF.Copy)
            outT = [op.tile([128, 2048], F32, tag=f"oT{i}", name=f"oT{i}")
                    for i in range(2)]
            for m in range(2):
                for n in range(2048 // 512):
                    ps = psum.tile([128, 512], F32, tag="mm", name="mm")
                    for k in range(2):
                        nc.tensor.matmul(
                            ps[:], wb[k][:, WB["Wm"] + m * 128:WB["Wm"] + (m + 1) * 128],
                            featT[k][:, n * 512:(n + 1) * 512],
                            start=(k == 0), stop=(k == 1))
                    nc.scalar.activation(outT[m][:, n * 512:(n + 1) * 512], ps[:],
                                         AF.Identity, bias=bias(f"bm{m}"),
                                         scale=1.0)
                nc.sync.dma_start(d_out[m], outT[m][:])

    nc.compile()
    return nc


def _prep_inputs(query, keys, ref_point, Wq, bq, Wk, bk, Woff, boff, WA, bA, Wm, bm):
    wl_arr = np.zeros(64, np.float32)
    hl_arr = np.zeros(64, np.float32)
    for h in range(HL):
        for s in range(SCALES):
            for k in range(KPTS):
                hl_, wl_ = HW_SIZES[s]
                wl_arr[h * 16 + s * 4 + k] = wl_
                hl_arr[h * 16 + s * 4 + k] = hl_
    cofx = wl_arr / (wl_arr - 1)
    cofy = hl_arr / (hl_arr - 1)

    rs = ref_point.reshape(Q, 2)
    refx = np.ascontiguousarray(rs[:, 0].reshape(QC, 128).T)
    refy = np.ascontiguousarray(rs[:, 1].reshape(QC, 128).T)

    cb16 = np.zeros((128, NCB), np.float32)
    cb16[:, CB["wlm1"]:CB["wlm1"] + 64] = wl_arr - 1
    cb16[:, CB["wlm2"]:CB["wlm2"] + 64] = wl_arr - 2
    cb16[:, CB["hlm1"]:CB["hlm1"] + 64] = hl_arr - 1
    cb16[:, CB["hlm2"]:CB["hlm2"] + 64] = hl_arr - 2
    cb16 = cb16.astype(BF)

    identb = np.eye(128, dtype=BF)
    pmat = np.zeros((128, 128), np.float32)
    for p in range(128):
        pmat[p, (p % 16) * 8 + p // 16] = 1.0

    in_maps = []
    for core in range(NCORES):
        b, hg = core // 2, core % 2
        heads = range(4 * hg, 4 * hg + 4)
        perm_off = np.zeros(128, np.int64)
        perm_A = np.zeros(64, np.int64)
        for i, h in enumerate(heads):
            for s in range(SCALES):
                for k in range(KPTS):
                    for xy in range(2):
                        perm_off[xy * 64 + i * 16 + s * 4 + k] = \
                            ((h * SCALES + s) * KPTS + k) * 2 + xy
                    perm_A[i * 16 + s * 4 + k] = (h * SCALES + s) * KPTS + k
        WoffP = Woff[:, perm_off]
        boffP = boff[perm_off]
        WAP = WA[:, perm_A]
        bAP = bA[perm_A]
        chs = slice(4 * hg * DK, (4 * hg + 4) * DK)

        wblob = np.concatenate(
            [Wq, Wk[:, chs], WoffP, WAP, Wm], axis=1)  # [256, 832]
        wblob = np.ascontiguousarray(
            wblob.reshape(2, 128, NWB).astype(BF))

        cblob = np.zeros((128, NCC), np.float32)
        cblob[:, CC["wl"]:CC["wl"] + 64] = wl_arr
        cblob[:, CC["hl"]:CC["hl"] + 64] = hl_arr
        cblob[:, CC["hlm1"]:CC["hlm1"] + 64] = hl_arr - 1
        cblob[:, CC["wlp1"]:CC["wlp1"] + 64] = wl_arr + 1
        cblob[:, CC["cofx"]:CC["cofx"] + 64] = cofx
        cblob[:, CC["cofy"]:CC["cofy"] + 64] = cofy
        cblob[:, CC["bcox"]:CC["bcox"] + 64] = boffP[0:64] * cofx - 0.5
        cblob[:, CC["bcoy"]:CC["bcoy"] + 64] = boffP[64:128] * cofy - 0.5
        cblob[:, CC["bA"]:CC["bA"] + 64] = bAP
        cblob[:, CC["refx"]:CC["refx"] + QC] = refx
        cblob[:, CC["refy"]:CC["refy"] + QC] = refy
        cblob[:, CC["bq0"]] = bq[0:128]
        cblob[:, CC["bq1"]] = bq[128:256]
        cblob[:, CC["bk"]] = bk[chs]
        cblob[:, CC["bm0"]] = bm[0:128]
        cblob[:, CC["bm1"]] = bm[128:256]
        cblob[:, CC["cneghalf"]] = -0.5
        cblob[:, CC["cone"]] = 1.0

        qs = query[b].reshape(Q, D)
        m = {
            "wblob": wblob,
            "cblob": np.ascontiguousarray(cblob),
            "cb16": np.ascontiguousarray(cb16),
            "identb": np.ascontiguousarray(identb),
            "pmat": np.ascontiguousarray(pmat),
            "queryT": np.ascontiguousarray(qs.T.astype(BF)).reshape(2, 128, Q),
        }
        for l in range(SCALES):
            m[f"keysT{l}"] = np.ascontiguousarray(
                keys[l][b].reshape(POS[l], D).T.astype(BF)).reshape(2, 128, POS[l])
        in_maps.append(m)
    return in_maps


def kernel(query, keys0, keys1, keys2, keys3, ref_point,
           Wq, bq, Wk, bk, Woff, boff, WA, bA, Wm, bm):
    query = np.asarray(query, np.float32)
    keys = [np.asarray(k, np.float32) for k in (keys0, keys1, keys2, keys3)]
    in_maps = _prep_inputs(
        query, keys, np.asarray(ref_point, np.float32),
        np.asarray(Wq, np.float32), np.asarray(bq, np.float32),
        np.asarray(Wk, np.float32), np.asarray(bk, np.float32),
        np.asarray(Woff, np.float32), np.asarray(boff, np.float32),
        np.asarray(WA, np.float32), np.asarray(bA, np.float32),
        np.asarray(Wm, np.float32), np.asarray(bm, np.float32))
    if "nc" not in _cache:
        _cache["nc"] = _build()
    nc = _cache["nc"]
    res = run_bass_kernel_spmd(nc, in_maps, list(range(NCORES)))
    out = np.zeros((B, H, W, D), np.float32)
    for core in range(NCORES):
        b, hg = core // 2, core % 2
        oT = res.results[core]["outT"].reshape(D, 2048)
        out[b, 32 * hg:32 * hg + 32] = oT.T.reshape(32, W, D)
    return out
